# revision 3
# baseline (speedup 1.0000x reference)
"""Trainium2 Bass kernel for nn_AutoregressiveLSA — v2 (quarter-split).

Reference math (complex, per batch b):
    Q  = WKQ @ E                      [2d, T]
    S  = E^H @ Q, keep i <= j         [T, T]
    out= WPV @ (E @ S) / rho_j        [d, T], cols 1..T-2 returned

v2 decomposition: split T into 4 quarters of L=512. With PT = (WPV E)^T:
    outT[j] = sum_{i<=j} S[i,j] PT[i]
            = Q[:,j]^T H_{q-1}  +  sum_{i in quarter(j), i<=j} S[i,j] PT[i]
    H_q = sum_{quarters a<=q} conj(E_a) @ PT_a        [2d, d]  (rank-accum)
which removes the inter-quarter portion of the score matrix (~40% of the
baseline's matmul cycles for phases B+C) and keeps every intermediate in
SBUF. All matmul operands are bf16 (same PE rate as f32r, half the DMA
and SBUF footprint); PSUM accumulation stays f32. Verified numerically:
rel err ~6.7e-3 vs the f32 reference (gate is 2e-2).

Karatsuba (3 real matmuls per complex product) everywhere:
  plain  a*b:      M1=ar·br M2=ai·bi M3=(ar+ai)(br+bi); Re=M1-M2, Im=M3-M1-M2
  conj(a)*b:       M1=ar·br M2=ai·bi M3=(ar-ai)(br+bi); Re=M1+M2, Im=M3-M1+M2

Sharding: data-parallel over batch, one NeuronCore per batch element.
The 1/rho_j scaling is applied at output evacuation on the Act engine
(per-partition scale, j is the partition dim there).
"""

import numpy as np

import concourse.bass as bass
import concourse.mybir as mybir
import concourse.tile as tile
from concourse import bacc
from concourse.bass_utils import run_bass_kernel_spmd

F32 = mybir.dt.float32
BF16 = mybir.dt.bfloat16
COPY = mybir.ActivationFunctionType.Copy

# Problem dims (hardcoded per contract)
B = 8
D2 = 1024   # 2*dim, channel dim of E
T = 2048    # sequence length
D = 512     # output channel dim
P = 128
L = 512     # quarter length
QN = T // L         # 4 quarters
KC = D2 // P        # 8 k-tiles over channel dim
MB = D2 // P        # 8 m-tiles for Q rows
TBQ = L // P        # 4 seq blocks per quarter
TB = T // P         # 16 seq blocks


def _mm(nc, out, lhsT, rhs, start, stop):
    nc.tensor.matmul(out, lhsT, rhs, start=start, stop=stop)


def build_module():
    nc = bacc.Bacc(target_bir_lowering=False, trn_type="TRN2")

    er_d = nc.dram_tensor("er", [D2, T], BF16, kind="ExternalInput")
    ei_d = nc.dram_tensor("ei", [D2, T], BF16, kind="ExternalInput")
    etr_d = nc.dram_tensor("etr", [QN - 1, TBQ, P, D2], BF16, kind="ExternalInput")
    eti_d = nc.dram_tensor("eti", [QN - 1, TBQ, P, D2], BF16, kind="ExternalInput")
    wtr_d = nc.dram_tensor("wtr", [MB, P, KC, P], BF16, kind="ExternalInput")
    wti_d = nc.dram_tensor("wti", [MB, P, KC, P], BF16, kind="ExternalInput")
    wts_d = nc.dram_tensor("wts", [MB, P, KC, P], BF16, kind="ExternalInput")
    wvr_d = nc.dram_tensor("wvr", [D2, D], BF16, kind="ExternalInput")
    wvi_d = nc.dram_tensor("wvi", [D2, D], BF16, kind="ExternalInput")
    wvs_d = nc.dram_tensor("wvs", [D2, D], BF16, kind="ExternalInput")
    mask_d = nc.dram_tensor("trimask", [P, P], BF16, kind="ExternalInput")
    rho_d = nc.dram_tensor("rho", [P, TB], F32, kind="ExternalInput")
    outT_re = nc.dram_tensor("outT_re", [T, D], F32, kind="ExternalOutput")
    outT_im = nc.dram_tensor("outT_im", [T, D], F32, kind="ExternalOutput")

    with tile.TileContext(nc) as tc:
        with tc.tile_pool(name="ps", bufs=2, space="PSUM") as ps, \
             tc.tile_pool(name="cst", bufs=1) as cst, \
             tc.tile_pool(name="hp", bufs=1) as hp, \
             tc.tile_pool(name="ep", bufs=2) as ep, \
             tc.tile_pool(name="edp", bufs=1) as edp, \
             tc.tile_pool(name="qp", bufs=2) as qp, \
             tc.tile_pool(name="qsp", bufs=1) as qsp, \
             tc.tile_pool(name="etp", bufs=1) as etp, \
             tc.tile_pool(name="ptp", bufs=1) as ptp, \
             tc.tile_pool(name="sp", bufs=1) as sp, \
             tc.tile_pool(name="wtp", bufs=2) as wtp, \
             tc.tile_pool(name="ev", bufs=2) as ev:

            _ctr = [0]

            def psum3(width):
                _ctr[0] += 1
                n = _ctr[0]
                t = [f"p{(3 * n + k) % 4}" for k in range(3)]
                return (ps.tile([P, 512], F32, tag=t[0], name=f"pa{n}")[:, :width],
                        ps.tile([P, 512], F32, tag=t[1], name=f"pb{n}")[:, :width],
                        ps.tile([P, 512], F32, tag=t[2], name=f"pc{n}")[:, :width])

            # ---- persistent small tensors ----
            rho_sb = cst.tile([P, TB], F32, tag="rho")
            mask_sb = cst.tile([P, P], BF16, tag="mask")
            wvr_t = cst.tile([P, KC, D], BF16, tag="wvr")
            wvi_t = cst.tile([P, KC, D], BF16, tag="wvi")
            wvs_t = cst.tile([P, KC, D], BF16, tag="wvs")
            nc.gpsimd.dma_start(rho_sb[:], rho_d[:])
            nc.gpsimd.dma_start(mask_sb[:], mask_d[:])
            nc.gpsimd.dma_start(wvr_t[:], wvr_d[:].rearrange("(kc p) d -> p kc d", p=P))
            nc.gpsimd.dma_start(wvi_t[:], wvi_d[:].rearrange("(kc p) d -> p kc d", p=P))
            nc.gpsimd.dma_start(wvs_t[:], wvs_d[:].rearrange("(kc p) d -> p kc d", p=P))

            # cumulative H (bf16 accumulators, + Hs = Hr+Hi)
            hr_t = hp.tile([P, KC, D], BF16, tag="hr")
            hi_t = hp.tile([P, KC, D], BF16, tag="hi")
            hs_t = hp.tile([P, KC, D], BF16, tag="hs")

            def load_E(q):
                js = bass.ds(q * L, L)
                er_t = ep.tile([P, KC, L], BF16, tag="er", name=f"er{q}")
                ei_t = ep.tile([P, KC, L], BF16, tag="ei", name=f"ei{q}")
                es_t = ep.tile([P, KC, L], BF16, tag="es", name=f"es{q}")
                nc.sync.dma_start(er_t[:], er_d[:, js].rearrange("(kc p) t -> p kc t", p=P))
                nc.sync.dma_start(ei_t[:], ei_d[:, js].rearrange("(kc p) t -> p kc t", p=P))
                nc.vector.tensor_add(es_t[:], er_t[:], ei_t[:])
                return er_t, ei_t, es_t

            def emit_A1(qq, E, Q):
                """Q(qq) = WKQ @ E(qq): per m-tile, karatsuba over kc."""
                er_t, ei_t, es_t = E
                qr_t, qi_t = Q
                for m in range(MB):
                    wtr_m = wtp.tile([P, KC, P], BF16, tag="wtr", name=f"wtr{qq}_{m}")
                    wti_m = wtp.tile([P, KC, P], BF16, tag="wti", name=f"wti{qq}_{m}")
                    wts_m = wtp.tile([P, KC, P], BF16, tag="wts", name=f"wts{qq}_{m}")
                    nc.gpsimd.dma_start(wtr_m[:], wtr_d[m])
                    nc.gpsimd.dma_start(wti_m[:], wti_d[m])
                    nc.gpsimd.dma_start(wts_m[:], wts_d[m])
                    pa, pb, pc = psum3(L)
                    for kc in range(KC):
                        first, last = kc == 0, kc == KC - 1
                        _mm(nc, pa, wtr_m[:, kc], er_t[:, kc], first, last)
                        _mm(nc, pb, wti_m[:, kc], ei_t[:, kc], first, last)
                        _mm(nc, pc, wts_m[:, kc], es_t[:, kc], first, last)
                    nc.any.tensor_copy(out=qr_t[:, m], in_=pa)
                    nc.vector.tensor_sub(qr_t[:, m], qr_t[:, m], pb)
                    nc.any.tensor_copy(out=qi_t[:, m], in_=pc)
                    nc.vector.tensor_sub(qi_t[:, m], qi_t[:, m], pa)
                    nc.vector.tensor_sub(qi_t[:, m], qi_t[:, m], pb)

            # ---- prologue: quarter 0 inputs + A1(0) ----
            E_cur = load_E(0)
            qr_cur = qp.tile([P, MB, L], BF16, tag="qr", name="qr0")
            qi_cur = qp.tile([P, MB, L], BF16, tag="qi", name="qi0")
            emit_A1(0, E_cur, (qr_cur, qi_cur))

            for q in range(QN):
                # ---- stage inputs for q+1 / this quarter's ET ----
                if q + 1 < QN:
                    E_nxt = load_E(q + 1)
                    qr_nxt = qp.tile([P, MB, L], BF16, tag="qr", name=f"qr{q+1}")
                    qi_nxt = qp.tile([P, MB, L], BF16, tag="qi", name=f"qi{q+1}")
                if q < QN - 1:
                    etr_t = etp.tile([P, TBQ, D2], BF16, tag="etr", name=f"etr{q}")
                    eti_t = etp.tile([P, TBQ, D2], BF16, tag="eti", name=f"eti{q}")
                    etd_t = etp.tile([P, TBQ, D2], BF16, tag="etd", name=f"etd{q}")
                    nc.gpsimd.dma_start(etr_t[:], etr_d[q].rearrange("tb p c -> p tb c"))
                    nc.gpsimd.dma_start(eti_t[:], eti_d[q].rearrange("tb p c -> p tb c"))
                    nc.vector.tensor_sub(etd_t[:], etr_t[:], eti_t[:])

                er_t, ei_t, es_t = E_cur
                ed_t = edp.tile([P, KC, L], BF16, tag="ed", name=f"ed{q}")
                nc.vector.tensor_sub(ed_t[:], er_t[:], ei_t[:])
                qs_t = qsp.tile([P, MB, L], BF16, tag="qs", name=f"qs{q}")
                nc.vector.tensor_add(qs_t[:], qr_cur[:], qi_cur[:])

                # ---- A2(q): PT = E_q^T WV^T  [4 x [P, D] blocks] ----
                ptr_t = ptp.tile([P, TBQ, D], BF16, tag="ptr", name=f"ptr{q}")
                pti_t = ptp.tile([P, TBQ, D], BF16, tag="pti", name=f"pti{q}")
                pts_t = ptp.tile([P, TBQ, D], BF16, tag="pts", name=f"pts{q}")
                for tb in range(TBQ):
                    tbs = bass.ds(tb * P, P)
                    pa, pb, pc = psum3(D)
                    for kc in range(KC):
                        first, last = kc == 0, kc == KC - 1
                        _mm(nc, pa, er_t[:, kc, tbs], wvr_t[:, kc], first, last)
                        _mm(nc, pb, ei_t[:, kc, tbs], wvi_t[:, kc], first, last)
                        _mm(nc, pc, es_t[:, kc, tbs], wvs_t[:, kc], first, last)
                    nc.any.tensor_copy(out=ptr_t[:, tb], in_=pa)
                    nc.vector.tensor_sub(ptr_t[:, tb], ptr_t[:, tb], pb)
                    nc.any.tensor_copy(out=pti_t[:, tb], in_=pc)
                    nc.vector.tensor_sub(pti_t[:, tb], pti_t[:, tb], pa)
                    nc.vector.tensor_sub(pti_t[:, tb], pti_t[:, tb], pb)
                nc.vector.tensor_add(pts_t[:], ptr_t[:], pti_t[:])

                # ---- triangle-B(q): S row-strips (conj karatsuba) ----
                srs, sis, sss = [], [], []
                for ib in range(TBQ):
                    W = L - ib * P
                    ibs = bass.ds(ib * P, P)
                    cs = bass.ds(ib * P, W)
                    pa, pb, pc = psum3(W)
                    for kc in range(KC):
                        first, last = kc == 0, kc == KC - 1
                        _mm(nc, pa, er_t[:, kc, ibs], qr_cur[:, kc, cs], first, last)
                        _mm(nc, pb, ei_t[:, kc, ibs], qi_cur[:, kc, cs], first, last)
                        _mm(nc, pc, ed_t[:, kc, ibs], qs_t[:, kc, cs], first, last)
                    sr = sp.tile([P, W], BF16, tag=f"sr{ib}", name=f"sr{q}_{ib}")
                    si = sp.tile([P, W], BF16, tag=f"si{ib}", name=f"si{q}_{ib}")
                    ss = sp.tile([P, W], BF16, tag=f"ss{ib}", name=f"ss{q}_{ib}")
                    nc.any.tensor_copy(out=sr[:], in_=pa)
                    nc.vector.tensor_add(sr[:], sr[:], pb)
                    nc.any.tensor_copy(out=si[:], in_=pc)
                    nc.vector.tensor_sub(si[:], si[:], pa)
                    nc.vector.tensor_add(si[:], si[:], pb)
                    dsl = bass.ds(0, P)  # diagonal block = first P cols of strip
                    nc.vector.tensor_mul(sr[:, dsl], sr[:, dsl], mask_sb[:])
                    nc.vector.tensor_mul(si[:, dsl], si[:, dsl], mask_sb[:])
                    nc.vector.tensor_add(ss[:], sr[:], si[:])
                    srs.append(sr); sis.append(si); sss.append(ss)

                # ---- apply(q) + triangle-C(q), fused PSUM accumulation ----
                for jb in range(TBQ):
                    jbs = bass.ds(jb * P, P)
                    pa, pb, pc = psum3(D)
                    first = True
                    if q > 0:
                        for kt in range(KC):
                            _mm(nc, pa, qr_cur[:, kt, jbs], hr_t[:, kt], kt == 0, False)
                            _mm(nc, pb, qi_cur[:, kt, jbs], hi_t[:, kt], kt == 0, False)
                            _mm(nc, pc, qs_t[:, kt, jbs], hs_t[:, kt], kt == 0, False)
                        first = False
                    for ib in range(jb + 1):
                        off = bass.ds((jb - ib) * P, P)
                        st = first and ib == 0
                        last = ib == jb
                        _mm(nc, pa, srs[ib][:, off], ptr_t[:, ib], st, last)
                        _mm(nc, pb, sis[ib][:, off], pti_t[:, ib], st, last)
                        _mm(nc, pc, sss[ib][:, off], pts_t[:, ib], st, last)
                    gjb = q * TBQ + jb
                    our = ev.tile([P, D], F32, tag="our", name=f"our{gjb}")
                    oui = ev.tile([P, D], F32, tag="oui", name=f"oui{gjb}")
                    rb = rho_sb[:, gjb : gjb + 1].to_broadcast([P, D])
                    nc.any.tensor_copy(out=our[:], in_=pa)
                    nc.vector.tensor_sub(our[:], our[:], pb)
                    nc.any.tensor_copy(out=oui[:], in_=pc)
                    nc.vector.tensor_sub(oui[:], oui[:], pa)
                    nc.vector.tensor_sub(oui[:], oui[:], pb)
                    nc.vector.tensor_mul(our[:], our[:], rb)
                    nc.vector.tensor_mul(oui[:], oui[:], rb)
                    nc.sync.dma_start(outT_re[bass.ts(gjb, P), :], our[:])
                    nc.sync.dma_start(outT_im[bass.ts(gjb, P), :], oui[:])

                # ---- H-update(q): H += conj(E_q) @ PT_q ----
                if q < QN - 1:
                    for mt in range(MB):
                        mts = bass.ts(mt, P)
                        pa, pb, pc = psum3(D)
                        for kt in range(TBQ):
                            first, last = kt == 0, kt == TBQ - 1
                            _mm(nc, pa, etr_t[:, kt, mts], ptr_t[:, kt], first, last)
                            _mm(nc, pb, eti_t[:, kt, mts], pti_t[:, kt], first, last)
                            _mm(nc, pc, etd_t[:, kt, mts], pts_t[:, kt], first, last)
                        if q == 0:
                            nc.any.tensor_copy(out=hr_t[:, mt], in_=pa)
                            nc.vector.tensor_add(hr_t[:, mt], hr_t[:, mt], pb)
                            nc.any.tensor_copy(out=hi_t[:, mt], in_=pc)
                            nc.vector.tensor_sub(hi_t[:, mt], hi_t[:, mt], pa)
                            nc.vector.tensor_add(hi_t[:, mt], hi_t[:, mt], pb)
                        else:
                            nc.vector.tensor_add(hr_t[:, mt], hr_t[:, mt], pa)
                            nc.vector.tensor_add(hr_t[:, mt], hr_t[:, mt], pb)
                            nc.vector.tensor_add(hi_t[:, mt], hi_t[:, mt], pc)
                            nc.vector.tensor_sub(hi_t[:, mt], hi_t[:, mt], pa)
                            nc.vector.tensor_add(hi_t[:, mt], hi_t[:, mt], pb)
                        nc.vector.tensor_add(hs_t[:, mt], hr_t[:, mt], hi_t[:, mt])

                # ---- A1(q+1) ----
                if q + 1 < QN:
                    emit_A1(q + 1, E_nxt, (qr_nxt, qi_nxt))
                    E_cur = E_nxt
                    qr_cur, qi_cur = qr_nxt, qi_nxt

    nc.compile()
    return nc


_NC_CACHE = None


def _get_module():
    global _NC_CACHE
    if _NC_CACHE is None:
        _NC_CACHE = build_module()
    return _NC_CACHE


def prep_shared(WKQ_re, WKQ_im, WPV_re, WPV_im):
    """Host-side weight prep, shared across cores (bf16)."""
    import ml_dtypes
    bft = ml_dtypes.bfloat16

    def blk(w):  # WKQ^T blocked for per-m lhsT streaming
        wt = np.ascontiguousarray(w.T)            # [c, c']
        return np.ascontiguousarray(
            wt.reshape(KC, P, MB, P).transpose(2, 1, 0, 3)).astype(bft)

    shared = {
        "wtr": blk(WKQ_re),
        "wti": blk(WKQ_im),
        "wts": blk(WKQ_re + WKQ_im),
        "wvr": np.ascontiguousarray(WPV_re.T).astype(bft),  # [c, d]
        "wvi": np.ascontiguousarray(WPV_im.T).astype(bft),
        "wvs": np.ascontiguousarray((WPV_re + WPV_im).T).astype(bft),
        "trimask": np.triu(np.ones((P, P), np.float32)).astype(bft),
    }
    j = np.arange(T, dtype=np.float32)
    rho = 1.0 / np.maximum(j, 1.0)
    shared["rho"] = np.ascontiguousarray(rho.reshape(TB, P).T)  # [p, jb]
    return shared


def kernel(E_re, E_im, WKQ_re, WKQ_im, WPV_re, WPV_im):
    import ml_dtypes
    bft = ml_dtypes.bfloat16
    E_re = np.asarray(E_re, dtype=np.float32)
    E_im = np.asarray(E_im, dtype=np.float32)
    shared = prep_shared(np.asarray(WKQ_re, np.float32),
                         np.asarray(WKQ_im, np.float32),
                         np.asarray(WPV_re, np.float32),
                         np.asarray(WPV_im, np.float32))
    in_maps = []
    for b in range(B):
        m = dict(shared)
        erb = E_re[b].astype(bft)
        eib = E_im[b].astype(bft)
        m["er"] = erb
        m["ei"] = eib
        # E^T quarters 0..2, blocked [q, tb, p, c]
        m["etr"] = np.ascontiguousarray(
            erb.T[: (QN - 1) * L].reshape(QN - 1, TBQ, P, D2))
        m["eti"] = np.ascontiguousarray(
            eib.T[: (QN - 1) * L].reshape(QN - 1, TBQ, P, D2))
        in_maps.append(m)

    nc = _get_module()
    res = run_bass_kernel_spmd(nc, in_maps, core_ids=list(range(B)))

    out = np.empty((B, D, T - 2), dtype=np.complex64)
    for b in range(B):
        r = res.results[b]["outT_re"]  # [T, D]
        i = res.results[b]["outT_im"]
        full = (r + 1j * i.astype(np.complex64)).T  # [D, T]
        out[b] = full[:, 1 : T - 1]
    return out


# revision 4
# speedup vs baseline: 1.0155x; 1.0155x over previous
"""Trainium2 Bass kernel for nn_AutoregressiveLSA — v2 (quarter-split).

Reference math (complex, per batch b):
    Q  = WKQ @ E                      [2d, T]
    S  = E^H @ Q, keep i <= j         [T, T]
    out= WPV @ (E @ S) / rho_j        [d, T], cols 1..T-2 returned

v2 decomposition: split T into 4 quarters of L=512. With PT = (WPV E)^T:
    outT[j] = sum_{i<=j} S[i,j] PT[i]
            = Q[:,j]^T H_{q-1}  +  sum_{i in quarter(j), i<=j} S[i,j] PT[i]
    H_q = sum_{quarters a<=q} conj(E_a) @ PT_a        [2d, d]  (rank-accum)
which removes the inter-quarter portion of the score matrix (~40% of the
baseline's matmul cycles for phases B+C) and keeps every intermediate in
SBUF. All matmul operands are bf16 (same PE rate as f32r, half the DMA
and SBUF footprint); PSUM accumulation stays f32. Verified numerically:
rel err ~6.7e-3 vs the f32 reference (gate is 2e-2).

Karatsuba (3 real matmuls per complex product) everywhere:
  plain  a*b:      M1=ar·br M2=ai·bi M3=(ar+ai)(br+bi); Re=M1-M2, Im=M3-M1-M2
  conj(a)*b:       M1=ar·br M2=ai·bi M3=(ar-ai)(br+bi); Re=M1+M2, Im=M3-M1+M2

Sharding: data-parallel over batch, one NeuronCore per batch element.
The 1/rho_j scaling is applied at output evacuation on the Act engine
(per-partition scale, j is the partition dim there).
"""

import numpy as np

import concourse.bass as bass
import concourse.mybir as mybir
import concourse.tile as tile
from concourse import bacc
from concourse.bass_utils import run_bass_kernel_spmd

F32 = mybir.dt.float32
BF16 = mybir.dt.bfloat16
COPY = mybir.ActivationFunctionType.Copy

# Problem dims (hardcoded per contract)
B = 8
D2 = 1024   # 2*dim, channel dim of E
T = 2048    # sequence length
D = 512     # output channel dim
P = 128
L = 512     # quarter length
QN = T // L         # 4 quarters
KC = D2 // P        # 8 k-tiles over channel dim
MB = D2 // P        # 8 m-tiles for Q rows
TBQ = L // P        # 4 seq blocks per quarter
TB = T // P         # 16 seq blocks


def _mm(nc, out, lhsT, rhs, start, stop):
    nc.tensor.matmul(out, lhsT, rhs, start=start, stop=stop)


def build_module():
    nc = bacc.Bacc(target_bir_lowering=False, trn_type="TRN2")

    er_d = nc.dram_tensor("er", [D2, T], BF16, kind="ExternalInput")
    ei_d = nc.dram_tensor("ei", [D2, T], BF16, kind="ExternalInput")
    es_d = nc.dram_tensor("es", [D2, T], BF16, kind="ExternalInput")
    ed_d = nc.dram_tensor("ed", [D2, T], BF16, kind="ExternalInput")
    etr_d = nc.dram_tensor("etr", [QN - 1, TBQ, P, D2], BF16, kind="ExternalInput")
    eti_d = nc.dram_tensor("eti", [QN - 1, TBQ, P, D2], BF16, kind="ExternalInput")
    etd_d = nc.dram_tensor("etd", [QN - 1, TBQ, P, D2], BF16, kind="ExternalInput")
    wtr_d = nc.dram_tensor("wtr", [MB, P, KC, P], BF16, kind="ExternalInput")
    wti_d = nc.dram_tensor("wti", [MB, P, KC, P], BF16, kind="ExternalInput")
    wts_d = nc.dram_tensor("wts", [MB, P, KC, P], BF16, kind="ExternalInput")
    wvr_d = nc.dram_tensor("wvr", [D2, D], BF16, kind="ExternalInput")
    wvi_d = nc.dram_tensor("wvi", [D2, D], BF16, kind="ExternalInput")
    wvs_d = nc.dram_tensor("wvs", [D2, D], BF16, kind="ExternalInput")
    mask_d = nc.dram_tensor("trimask", [P, P], BF16, kind="ExternalInput")
    rho_d = nc.dram_tensor("rho", [P, TB], F32, kind="ExternalInput")
    outT_re = nc.dram_tensor("outT_re", [T, D], F32, kind="ExternalOutput")
    outT_im = nc.dram_tensor("outT_im", [T, D], F32, kind="ExternalOutput")

    with tile.TileContext(nc) as tc:
        with tc.tile_pool(name="ps", bufs=2, space="PSUM") as ps, \
             tc.tile_pool(name="cst", bufs=1) as cst, \
             tc.tile_pool(name="hp", bufs=1) as hp, \
             tc.tile_pool(name="ep", bufs=2) as ep, \
             tc.tile_pool(name="edp", bufs=1) as edp, \
             tc.tile_pool(name="qp", bufs=2) as qp, \
             tc.tile_pool(name="qsp", bufs=1) as qsp, \
             tc.tile_pool(name="etp", bufs=1) as etp, \
             tc.tile_pool(name="ptp", bufs=1) as ptp, \
             tc.tile_pool(name="sp", bufs=1) as sp, \
             tc.tile_pool(name="wtp", bufs=2) as wtp, \
             tc.tile_pool(name="ev", bufs=2) as ev:

            _ctr = [0]

            def psum3(width):
                _ctr[0] += 1
                n = _ctr[0]
                t = [f"p{(3 * n + k) % 4}" for k in range(3)]
                return (ps.tile([P, 512], F32, tag=t[0], name=f"pa{n}")[:, :width],
                        ps.tile([P, 512], F32, tag=t[1], name=f"pb{n}")[:, :width],
                        ps.tile([P, 512], F32, tag=t[2], name=f"pc{n}")[:, :width])

            # ---- persistent small tensors (loads issued after A1(0)
            # emission so the wt tiles win the SWDGE queue at startup) ----
            rho_sb = cst.tile([P, TB], F32, tag="rho")
            mask_sb = cst.tile([P, P], BF16, tag="mask")
            wvr_t = cst.tile([P, KC, D], BF16, tag="wvr")
            wvi_t = cst.tile([P, KC, D], BF16, tag="wvi")
            wvs_t = cst.tile([P, KC, D], BF16, tag="wvs")

            # cumulative H (bf16 accumulators, + Hs = Hr+Hi)
            hr_t = hp.tile([P, KC, D], BF16, tag="hr")
            hi_t = hp.tile([P, KC, D], BF16, tag="hi")
            hs_t = hp.tile([P, KC, D], BF16, tag="hs")

            def load_E(q, per_kc=False):
                js = bass.ds(q * L, L)
                er_t = ep.tile([P, KC, L], BF16, tag="er", name=f"er{q}")
                ei_t = ep.tile([P, KC, L], BF16, tag="ei", name=f"ei{q}")
                es_t = ep.tile([P, KC, L], BF16, tag="es", name=f"es{q}")
                if per_kc:
                    # kc-major interleave so A1's first matmuls start after
                    # the first k-tile lands (startup critical path)
                    for kc in range(KC):
                        ks = bass.ts(kc, P)
                        nc.sync.dma_start(er_t[:, kc], er_d[ks, js])
                        nc.sync.dma_start(ei_t[:, kc], ei_d[ks, js])
                        nc.sync.dma_start(es_t[:, kc], es_d[ks, js])
                else:
                    nc.sync.dma_start(er_t[:], er_d[:, js].rearrange("(kc p) t -> p kc t", p=P))
                    nc.sync.dma_start(ei_t[:], ei_d[:, js].rearrange("(kc p) t -> p kc t", p=P))
                    nc.sync.dma_start(es_t[:], es_d[:, js].rearrange("(kc p) t -> p kc t", p=P))
                return er_t, ei_t, es_t

            def emit_A1_m(qq, E, Q, m):
                """One m-tile of Q(qq) = WKQ @ E(qq), karatsuba over kc."""
                er_t, ei_t, es_t = E
                qr_t, qi_t = Q
                wtr_m = wtp.tile([P, KC, P], BF16, tag="wtr", name=f"wtr{qq}_{m}")
                wti_m = wtp.tile([P, KC, P], BF16, tag="wti", name=f"wti{qq}_{m}")
                wts_m = wtp.tile([P, KC, P], BF16, tag="wts", name=f"wts{qq}_{m}")
                nc.gpsimd.dma_start(wtr_m[:], wtr_d[m])
                nc.gpsimd.dma_start(wti_m[:], wti_d[m])
                nc.gpsimd.dma_start(wts_m[:], wts_d[m])
                pa, pb, pc = psum3(L)
                for kc in range(KC):
                    first, last = kc == 0, kc == KC - 1
                    _mm(nc, pa, wtr_m[:, kc], er_t[:, kc], first, last)
                    _mm(nc, pb, wti_m[:, kc], ei_t[:, kc], first, last)
                    _mm(nc, pc, wts_m[:, kc], es_t[:, kc], first, last)
                nc.any.tensor_copy(out=qr_t[:, m], in_=pa)
                nc.vector.tensor_sub(qr_t[:, m], qr_t[:, m], pb)
                nc.any.tensor_copy(out=qi_t[:, m], in_=pc)
                nc.vector.tensor_sub(qi_t[:, m], qi_t[:, m], pa)
                nc.vector.tensor_sub(qi_t[:, m], qi_t[:, m], pb)

            # ---- prologue: quarter 0 inputs + A1(0) ----
            E_cur = load_E(0, per_kc=True)
            qr_cur = qp.tile([P, MB, L], BF16, tag="qr", name="qr0")
            qi_cur = qp.tile([P, MB, L], BF16, tag="qi", name="qi0")
            for m in range(MB):
                emit_A1_m(0, E_cur, (qr_cur, qi_cur), m)
            nc.gpsimd.dma_start(rho_sb[:], rho_d[:])
            nc.gpsimd.dma_start(mask_sb[:], mask_d[:])
            nc.gpsimd.dma_start(wvr_t[:], wvr_d[:].rearrange("(kc p) d -> p kc d", p=P))
            nc.gpsimd.dma_start(wvi_t[:], wvi_d[:].rearrange("(kc p) d -> p kc d", p=P))
            nc.gpsimd.dma_start(wvs_t[:], wvs_d[:].rearrange("(kc p) d -> p kc d", p=P))

            for q in range(QN):
                # ---- stage inputs for q+1 / this quarter's ET ----
                if q + 1 < QN:
                    E_nxt = load_E(q + 1)
                    qr_nxt = qp.tile([P, MB, L], BF16, tag="qr", name=f"qr{q+1}")
                    qi_nxt = qp.tile([P, MB, L], BF16, tag="qi", name=f"qi{q+1}")
                if q < QN - 1:
                    etr_t = etp.tile([P, TBQ, D2], BF16, tag="etr", name=f"etr{q}")
                    eti_t = etp.tile([P, TBQ, D2], BF16, tag="eti", name=f"eti{q}")
                    etd_t = etp.tile([P, TBQ, D2], BF16, tag="etd", name=f"etd{q}")
                    nc.gpsimd.dma_start(etr_t[:], etr_d[q].rearrange("tb p c -> p tb c"))
                    nc.gpsimd.dma_start(eti_t[:], eti_d[q].rearrange("tb p c -> p tb c"))
                    nc.gpsimd.dma_start(etd_t[:], etd_d[q].rearrange("tb p c -> p tb c"))

                er_t, ei_t, es_t = E_cur
                ed_t = edp.tile([P, KC, L], BF16, tag="ed", name=f"ed{q}")
                nc.sync.dma_start(
                    ed_t[:], ed_d[:, bass.ds(q * L, L)].rearrange("(kc p) t -> p kc t", p=P))
                qs_t = qsp.tile([P, MB, L], BF16, tag="qs", name=f"qs{q}")
                nc.vector.tensor_add(qs_t[:], qr_cur[:], qi_cur[:])

                # ---- A2(q): PT = E_q^T WV^T  [4 x [P, D] blocks] ----
                ptr_t = ptp.tile([P, TBQ, D], BF16, tag="ptr", name=f"ptr{q}")
                pti_t = ptp.tile([P, TBQ, D], BF16, tag="pti", name=f"pti{q}")
                pts_t = ptp.tile([P, TBQ, D], BF16, tag="pts", name=f"pts{q}")
                for tb in range(TBQ):
                    tbs = bass.ds(tb * P, P)
                    pa, pb, pc = psum3(D)
                    for kc in range(KC):
                        first, last = kc == 0, kc == KC - 1
                        _mm(nc, pa, er_t[:, kc, tbs], wvr_t[:, kc], first, last)
                        _mm(nc, pb, ei_t[:, kc, tbs], wvi_t[:, kc], first, last)
                        _mm(nc, pc, es_t[:, kc, tbs], wvs_t[:, kc], first, last)
                    nc.any.tensor_copy(out=ptr_t[:, tb], in_=pa)
                    nc.vector.tensor_sub(ptr_t[:, tb], ptr_t[:, tb], pb)
                    nc.any.tensor_copy(out=pti_t[:, tb], in_=pc)
                    nc.vector.tensor_sub(pti_t[:, tb], pti_t[:, tb], pa)
                    nc.vector.tensor_sub(pti_t[:, tb], pti_t[:, tb], pb)
                nc.vector.tensor_add(pts_t[:], ptr_t[:], pti_t[:])

                # ---- triangle-B(q): S row-strips (conj karatsuba) ----
                srs, sis, sss = [], [], []
                for ib in range(TBQ):
                    W = L - ib * P
                    ibs = bass.ds(ib * P, P)
                    cs = bass.ds(ib * P, W)
                    pa, pb, pc = psum3(W)
                    for kc in range(KC):
                        first, last = kc == 0, kc == KC - 1
                        _mm(nc, pa, er_t[:, kc, ibs], qr_cur[:, kc, cs], first, last)
                        _mm(nc, pb, ei_t[:, kc, ibs], qi_cur[:, kc, cs], first, last)
                        _mm(nc, pc, ed_t[:, kc, ibs], qs_t[:, kc, cs], first, last)
                    sr = sp.tile([P, W], BF16, tag=f"sr{ib}", name=f"sr{q}_{ib}")
                    si = sp.tile([P, W], BF16, tag=f"si{ib}", name=f"si{q}_{ib}")
                    ss = sp.tile([P, W], BF16, tag=f"ss{ib}", name=f"ss{q}_{ib}")
                    nc.any.tensor_copy(out=sr[:], in_=pa)
                    nc.vector.tensor_add(sr[:], sr[:], pb)
                    nc.any.tensor_copy(out=si[:], in_=pc)
                    nc.vector.tensor_sub(si[:], si[:], pa)
                    nc.vector.tensor_add(si[:], si[:], pb)
                    dsl = bass.ds(0, P)  # diagonal block = first P cols of strip
                    nc.vector.tensor_mul(sr[:, dsl], sr[:, dsl], mask_sb[:])
                    nc.vector.tensor_mul(si[:, dsl], si[:, dsl], mask_sb[:])
                    nc.vector.tensor_add(ss[:], sr[:], si[:])
                    srs.append(sr); sis.append(si); sss.append(ss)

                # ---- apply(q) + triangle-C(q), fused PSUM accumulation ----
                for jb in range(TBQ):
                    jbs = bass.ds(jb * P, P)
                    pa, pb, pc = psum3(D)
                    first = True
                    if q > 0:
                        for kt in range(KC):
                            _mm(nc, pa, qr_cur[:, kt, jbs], hr_t[:, kt], kt == 0, False)
                            _mm(nc, pb, qi_cur[:, kt, jbs], hi_t[:, kt], kt == 0, False)
                            _mm(nc, pc, qs_t[:, kt, jbs], hs_t[:, kt], kt == 0, False)
                        first = False
                    for ib in range(jb + 1):
                        off = bass.ds((jb - ib) * P, P)
                        st = first and ib == 0
                        last = ib == jb
                        _mm(nc, pa, srs[ib][:, off], ptr_t[:, ib], st, last)
                        _mm(nc, pb, sis[ib][:, off], pti_t[:, ib], st, last)
                        _mm(nc, pc, sss[ib][:, off], pts_t[:, ib], st, last)
                    gjb = q * TBQ + jb
                    our = ev.tile([P, D], F32, tag="our", name=f"our{gjb}")
                    oui = ev.tile([P, D], F32, tag="oui", name=f"oui{gjb}")
                    rb = rho_sb[:, gjb : gjb + 1].to_broadcast([P, D])
                    nc.any.tensor_copy(out=our[:], in_=pa)
                    nc.vector.tensor_sub(our[:], our[:], pb)
                    nc.any.tensor_copy(out=oui[:], in_=pc)
                    nc.vector.tensor_sub(oui[:], oui[:], pa)
                    nc.vector.tensor_sub(oui[:], oui[:], pb)
                    nc.vector.tensor_mul(our[:], our[:], rb)
                    nc.vector.tensor_mul(oui[:], oui[:], rb)
                    nc.sync.dma_start(outT_re[bass.ts(gjb, P), :], our[:])
                    nc.sync.dma_start(outT_im[bass.ts(gjb, P), :], oui[:])

                # ---- H-update(q) interleaved with A1(q+1), m-tile by
                # m-tile: the H evacuation is DVE-heavy (6 TT per m-tile
                # vs only ~2.5us of PE work), so alternating with A1's
                # 5us m-tile groups keeps the PE fed while DVE drains ----
                if q < QN - 1:
                    for mt in range(MB):
                        mts = bass.ts(mt, P)
                        pa, pb, pc = psum3(D)
                        for kt in range(TBQ):
                            first, last = kt == 0, kt == TBQ - 1
                            _mm(nc, pa, etr_t[:, kt, mts], ptr_t[:, kt], first, last)
                            _mm(nc, pb, eti_t[:, kt, mts], pti_t[:, kt], first, last)
                            _mm(nc, pc, etd_t[:, kt, mts], pts_t[:, kt], first, last)
                        if q == 0:
                            nc.any.tensor_copy(out=hr_t[:, mt], in_=pa)
                            nc.vector.tensor_add(hr_t[:, mt], hr_t[:, mt], pb)
                            nc.any.tensor_copy(out=hi_t[:, mt], in_=pc)
                            nc.vector.tensor_sub(hi_t[:, mt], hi_t[:, mt], pa)
                            nc.vector.tensor_add(hi_t[:, mt], hi_t[:, mt], pb)
                        else:
                            nc.vector.tensor_add(hr_t[:, mt], hr_t[:, mt], pa)
                            nc.vector.tensor_add(hr_t[:, mt], hr_t[:, mt], pb)
                            nc.vector.tensor_add(hi_t[:, mt], hi_t[:, mt], pc)
                            nc.vector.tensor_sub(hi_t[:, mt], hi_t[:, mt], pa)
                            nc.vector.tensor_add(hi_t[:, mt], hi_t[:, mt], pb)
                        nc.vector.tensor_add(hs_t[:, mt], hr_t[:, mt], hi_t[:, mt])
                        emit_A1_m(q + 1, E_nxt, (qr_nxt, qi_nxt), mt)
                    E_cur = E_nxt
                    qr_cur, qi_cur = qr_nxt, qi_nxt

    nc.compile()
    return nc


_NC_CACHE = None


def _get_module():
    global _NC_CACHE
    if _NC_CACHE is None:
        _NC_CACHE = build_module()
    return _NC_CACHE


def prep_shared(WKQ_re, WKQ_im, WPV_re, WPV_im):
    """Host-side weight prep, shared across cores (bf16)."""
    import ml_dtypes
    bft = ml_dtypes.bfloat16

    def blk(w):  # WKQ^T blocked for per-m lhsT streaming
        wt = np.ascontiguousarray(w.T)            # [c, c']
        return np.ascontiguousarray(
            wt.reshape(KC, P, MB, P).transpose(2, 1, 0, 3)).astype(bft)

    shared = {
        "wtr": blk(WKQ_re),
        "wti": blk(WKQ_im),
        "wts": blk(WKQ_re + WKQ_im),
        "wvr": np.ascontiguousarray(WPV_re.T).astype(bft),  # [c, d]
        "wvi": np.ascontiguousarray(WPV_im.T).astype(bft),
        "wvs": np.ascontiguousarray((WPV_re + WPV_im).T).astype(bft),
        "trimask": np.triu(np.ones((P, P), np.float32)).astype(bft),
    }
    j = np.arange(T, dtype=np.float32)
    rho = 1.0 / np.maximum(j, 1.0)
    shared["rho"] = np.ascontiguousarray(rho.reshape(TB, P).T)  # [p, jb]
    return shared


def kernel(E_re, E_im, WKQ_re, WKQ_im, WPV_re, WPV_im):
    import ml_dtypes
    bft = ml_dtypes.bfloat16
    E_re = np.asarray(E_re, dtype=np.float32)
    E_im = np.asarray(E_im, dtype=np.float32)
    shared = prep_shared(np.asarray(WKQ_re, np.float32),
                         np.asarray(WKQ_im, np.float32),
                         np.asarray(WPV_re, np.float32),
                         np.asarray(WPV_im, np.float32))
    in_maps = []
    for b in range(B):
        m = dict(shared)
        erb = E_re[b].astype(bft)
        eib = E_im[b].astype(bft)
        esb = (E_re[b] + E_im[b]).astype(bft)
        edb = (E_re[b] - E_im[b]).astype(bft)
        m["er"] = erb
        m["ei"] = eib
        m["es"] = esb
        m["ed"] = edb
        # E^T quarters 0..2, blocked [q, tb, p, c]
        def tq(x):
            return np.ascontiguousarray(
                x.T[: (QN - 1) * L].reshape(QN - 1, TBQ, P, D2))
        m["etr"] = tq(erb)
        m["eti"] = tq(eib)
        m["etd"] = tq(edb)
        in_maps.append(m)

    nc = _get_module()
    res = run_bass_kernel_spmd(nc, in_maps, core_ids=list(range(B)))

    out = np.empty((B, D, T - 2), dtype=np.complex64)
    for b in range(B):
        r = res.results[b]["outT_re"]  # [T, D]
        i = res.results[b]["outT_im"]
        full = (r + 1j * i.astype(np.complex64)).T  # [D, T]
        out[b] = full[:, 1 : T - 1]
    return out
